# revision 73
# baseline (speedup 1.0000x reference)
"""8-core TP attention kernel for Trainium2 (Bass/Tile).

Problem: B=2, T=S=2048, D=2048, N=16 q-heads, KH=8 kv-heads, H=128.
Sharding: TP over heads. Core c owns q-heads {2c, 2c+1}, kv-head c, and the
D-output slice [256c, 256(c+1)) of o_proj. Per-head attention outputs are
AllGathered (bf16) across cores; o_proj is sharded on its output dim so the
host just concatenates the 8 output slices.

Changes vs the 485us fp32r baseline (now ~390-400us measured):
- bf16 end-to-end on the matmul path (x, qkv weights, kT/qT/vS, exp, mask):
  same PE stream rate as fp32r but ~2x faster LDWEIGHTS and half the x DMA.
- softmax sums no longer burn a PE stream per block: exp tiles accumulate on
  DVE (esum, f32r) and a single ones-matmul per (b,chunk,g) reduces across
  partitions.
- the causal mask is a 0/1 multiply on the exp output (bf16, DVE) instead of
  a K_MASK bias add on the PSUM logits.
- three parallel startup DMA streams: qkv weights on sync, x tiles on
  gpsimd, rope/bias on scalar; o_w loads after b0's sections. w/rope/o_w
  use per-partition-contiguous DRAM layouts (one fat descriptor per
  partition instead of thousands of scattered ~1KB reads), which removed
  the ~16us startup stall entirely.
- b1's projection is split 12+4 with attention chunks 0-2 in between, so
  their AllGathers drain ~60us earlier on the serialized CC pipe and overlap
  compute instead of forming the tail. SBUF pools are shared across the
  whole kernel (continuous x-tile prefetch); only PSUM pools are scoped per
  section. o_proj chunks for b0 fill section boundaries; the rest drain at
  the end in AllGather-completion order, covering the last AllGather.
"""
import sys
import os

sys.path.insert(0, "/opt/trn_rl_repo")

# Provide antenv.axon_hooks (missing from the read-only antenv package on
# PYTHONPATH) so bass_utils can capture NTFF profiles under axon when
# trace=True. Degrades to a None hook (trace skipped) when the .so lacks the
# profile symbols.
if "antenv.axon_hooks" not in sys.modules:
    import types as _types

    _mod = _types.ModuleType("antenv.axon_hooks")

    def _default_ntff_hook():
        import contextlib
        import ctypes

        so_path = "/opt/axon/libaxon_pjrt.so"
        if not os.path.exists(so_path):
            return None
        lib = ctypes.CDLL(so_path)
        if not hasattr(lib, "axon_start_nrt_profile"):
            return None
        lib.axon_start_nrt_profile.argtypes = [
            ctypes.POINTER(ctypes.c_int64), ctypes.c_size_t]
        lib.axon_start_nrt_profile.restype = ctypes.c_int64
        lib.axon_stop_nrt_profile.argtypes = [ctypes.c_char_p]
        lib.axon_stop_nrt_profile.restype = ctypes.c_int64

        @contextlib.contextmanager
        def _hook(output_dir, device_ids):
            import jax
            jax.devices()
            if device_ids:
                ids = (ctypes.c_int64 * len(device_ids))(*device_ids)
                rc = lib.axon_start_nrt_profile(ids, len(device_ids))
            else:
                rc = lib.axon_start_nrt_profile(None, 0)
            if rc != 0:
                raise RuntimeError(f"axon_start_nrt_profile rc={rc}")
            try:
                yield
            finally:
                n = lib.axon_stop_nrt_profile(str(output_dir).encode())
                if n < 0:
                    raise RuntimeError(f"axon_stop_nrt_profile rc={n}")
                print(f"profile: {n} file(s) written to {output_dir}")

        return _hook

    _mod._HOOK = None

    def _set_hook(hook, _m=_mod):
        _m._HOOK = hook

    def _get_hook(_m=_mod):
        if _m._HOOK is None:
            _m._HOOK = _default_ntff_hook()
        return _m._HOOK

    _mod.set_axon_ntff_profile_hook = _set_hook
    _mod.get_axon_ntff_profile_hook = _get_hook
    sys.modules["antenv.axon_hooks"] = _mod
    try:
        import antenv as _antenv
        _antenv.axon_hooks = _mod
    except ImportError:
        pass

import numpy as np

B, T, D = 2, 2048, 2048
N, KH, H = 16, 8, 128
S = 2048
EPS = 1e-6
ROPE_THETA = 1000000.0
K_MASK = -0.7 * float(np.finfo(np.float32).max)
NCORES = 8
GLOC = N // NCORES        # 2 local q heads
DLOC = D // NCORES        # 256 output cols per core
NTT = T // 128            # 16 t-tiles
NTC = T // 512            # 4 t-chunks
NDC = D // 128            # 16 d-chunks
NSB = S // 128            # 16 s-blocks
HSCALE = float(H) ** -0.5
MAX_BIAS = 8


def _num_left_pad(seg):
    return np.sum(np.cumsum(seg != 0, axis=-1) == 0, axis=-1).astype(np.int32)


def _positions_from_segment_ids(seg):
    t = seg.shape[1]
    pos = np.arange(t, dtype=np.int32)[None, :] - np.argmax(seg, axis=1)[:, None]
    return np.where(seg != 0, pos, 2 ** 30)


def _host_mask_and_rope(x, q_norm_w, k_norm_w, segment_ids, start_ind, cur_ind):
    """Reproduce the reference mask / positions / rope tables in numpy."""
    b, t = segment_ids.shape
    s = S
    start = np.where(start_ind < 0, _num_left_pad(segment_ids), start_ind).astype(np.int64)
    pos = _positions_from_segment_ids(segment_ids).astype(np.int64) + int(cur_ind)

    fraction = np.arange(0, H, 2, dtype=np.float32) / np.float32(H)
    inv_freq = (1.0 / (np.float32(ROPE_THETA) ** fraction)).astype(np.float32)
    sinusoid = (pos.astype(np.float32)[:, :, None] * inv_freq[None, None, :]).astype(np.float32)
    sin, cos = np.sin(sinusoid).astype(np.float32), np.cos(sinusoid).astype(np.float32)

    q_pos = int(cur_ind) + np.arange(t, dtype=np.int64)[None, :] - start[:, None]
    ts_ = np.arange(s, dtype=np.int64)
    kv_seg = (ts_[None, :] >= start[:, None]) & (ts_[None, :] < int(cur_ind) + t)
    k_pos = ts_[None, :] - start[:, None]
    causal = k_pos[:, None, :] <= q_pos[:, :, None]
    seg_mask = kv_seg[:, None, :].astype(segment_ids.dtype) == segment_ids[:, :, None]
    final_mask = causal & seg_mask  # [B, T, S]
    return final_mask, sin, cos


def _numpy_reference(x, q_w, k_w, v_w, o_w, q_norm_w, k_norm_w, k_cache, v_cache,
                     segment_ids, start_ind, cur_ind):
    """Exact-ish numpy fallback (used only for non-structural inputs)."""
    def rms_norm(v, w):
        rms = np.sqrt(np.mean(v.astype(np.float32) ** 2, axis=-1, keepdims=True) + EPS)
        return (w * v / rms).astype(v.dtype)

    mask, sin, cos = _host_mask_and_rope(x, q_norm_w, k_norm_w, segment_ids,
                                         start_ind, cur_ind)

    q = rms_norm(np.einsum('BTD,DNH->BTNH', x, q_w), q_norm_w)
    k = rms_norm(np.einsum('BSD,DKH->BSKH', x, k_w), k_norm_w)
    v = np.einsum('BSD,DKH->BSKH', x, v_w)

    def rope(z):
        h = z.shape[-1] // 2
        z1, z2 = z[..., :h], z[..., h:]
        s_, c_ = sin[:, :, None, :], cos[:, :, None, :]
        return np.concatenate([z1 * c_ - z2 * s_, z2 * c_ + z1 * s_], axis=-1).astype(z.dtype)

    q, k = rope(q), rope(k)
    kc = np.array(k_cache)
    vc = np.array(v_cache)
    ci = int(cur_ind)
    kc[:, ci:ci + T] = k
    vc[:, ci:ci + T] = v

    b, t = x.shape[0], x.shape[1]
    qg = q.reshape(b, t, KH, N // KH, H)
    logits = np.einsum('BTHGD,BSHD->BHGTS', qg, kc) * HSCALE
    logits = np.where(mask[:, None, None, :, :], logits, np.float32(K_MASK))
    m = logits.max(axis=-1, keepdims=True)
    e = np.exp(logits - m)
    attn = (e / e.sum(axis=-1, keepdims=True)).astype(np.float32)
    o = np.einsum('BHGTS,BSHD->BTHGD', attn, vc).reshape(b, t, N, H)
    return np.einsum('BTNH,NHD->BTD', o, o_w).astype(np.float32)


def _block_structure(mask):
    """Classify [s,t] blocks of 128x512 per (b, tc).

    Returns blocks[b][tc] = list of (sb, bias_idx|None) and bias tiles
    [nbias, 128, 512] f32 additive masks (0 valid / K_MASK invalid), or None
    if the structure is unsupported (fallback needed).
    """
    bias_tiles = []
    bias_map = {}
    blocks = []
    for b in range(B):
        per_b = []
        for tcc in range(NTC):
            sub = mask[b, tcc * 512:(tcc + 1) * 512, :]  # [512 t, S]
            lst = []
            for sb in range(NSB):
                blk = sub[:, sb * 128:(sb + 1) * 128].T  # [128 s, 512 t]
                if not blk.any():
                    continue
                if blk.all():
                    lst.append((sb, None))
                    continue
                key = blk.tobytes()
                if key not in bias_map:
                    bias_map[key] = len(bias_tiles)
                    bias_tiles.append(np.where(blk, np.float32(0), np.float32(K_MASK)))
                lst.append((sb, bias_map[key]))
            per_b.append(lst)
        blocks.append(per_b)
    if len(bias_tiles) == 0:
        bias_tiles.append(np.zeros((128, 512), np.float32))
    if len(bias_tiles) > MAX_BIAS:
        return None, None
    return blocks, np.stack(bias_tiles)


LAST_RES = None


def _build_and_run(xT_bf, w_all, o_w_bf, rope_sets, rope_idx, bias_np, blocks):
    import ml_dtypes
    import concourse.bass as bass
    import concourse.mybir as mybir
    import concourse.tile as tile
    from concourse import bacc
    from concourse.bass_utils import run_bass_kernel_spmd
    from concourse.masks import make_identity

    F32 = mybir.dt.float32
    F32R = mybir.dt.float32r
    BF16 = mybir.dt.bfloat16
    NBIAS = bias_np.shape[0]
    NRSETS = rope_sets.shape[1] // (4 * NTT * 64)

    nc = bacc.Bacc("TRN2", target_bir_lowering=False, debug=False, num_devices=NCORES)
    eps_t = nc.alloc_sbuf_tensor("const-eps", [128, 1], F32)
    nc.gpsimd.memset(eps_t.ap(), float(EPS))
    nc.const_aps.aps[(F32, float(EPS))] = eps_t.ap()

    # ---- external I/O ----
    xT_d = nc.dram_tensor("xT", [B, NTT, 128, NDC, 128], BF16, kind="ExternalInput").ap()
    w_d = nc.dram_tensor("w_all", [128, NDC * 512], BF16, kind="ExternalInput").ap()
    ow_d = nc.dram_tensor("o_w", [128, N * DLOC], BF16, kind="ExternalInput").ap()
    rope_d = nc.dram_tensor("rope", [128, NRSETS * 4 * NTT * 64], F32,
                            kind="ExternalInput").ap()
    bias_d = nc.dram_tensor("bias", [NBIAS, 128, 512], BF16, kind="ExternalInput").ap()
    out_d = nc.dram_tensor("out", [2, 128, B * T], F32, kind="ExternalOutput").ap()

    # ---- collective buffers: per (b,tc) chunks so the AllGather stream
    # starts as soon as the first chunk's attention lands and stays
    # pipelined with compute ----
    ag_in1 = [[nc.dram_tensor(f"agin_{b}_{tcc}", [GLOC * 128, 512], BF16)
               for tcc in range(NTC)] for b in range(B)]
    ag_out1 = [[nc.dram_tensor(f"agout_{b}_{tcc}", [N * 128, 512], BF16,
                               addr_space="Shared") for tcc in range(NTC)]
               for b in range(B)]

    with tile.TileContext(nc) as tc:
        with tc.tile_pool(name="const", bufs=1) as cpool, \
             tc.tile_pool(name="store", bufs=1) as spool:
            # persistent tiles; w on the sync queue, x tiles on the gpsimd
            # queue, rope/bias on the scalar queue — three parallel streams
            # so the first projection matmul starts ~15us in
            # per-partition-contiguous layout: 128 fat descriptors instead
            # of 2048 scattered 1KB reads, so w lands ~12us earlier
            w_sb = cpool.tile([128, NDC, 512], BF16, tag="w")
            nc.sync.dma_start(out=w_sb[:], in_=bass.AP(
                w_d.tensor, 0, [[NDC * 512, 128], [1, NDC * 512]]))
            ow_sb = cpool.tile([128, N, DLOC], BF16, tag="ow")
            rope_sb = cpool.tile([128, NRSETS, 4, NTT, 64], F32, tag="rope")
            nc.scalar.dma_start(out=rope_sb[:], in_=bass.AP(
                rope_d.tensor, 0,
                [[NRSETS * 4 * NTT * 64, 128], [1, NRSETS * 4 * NTT * 64]]))
            bias_sb = cpool.tile([128, NBIAS, 512], BF16, tag="bias")
            nc.scalar.dma_start(out=bias_sb[:], in_=bass.AP(
                bias_d.tensor, 0, [[512, 128], [128 * 512, NBIAS], [1, 512]]))
            ident = cpool.tile([128, 128], BF16, tag="ident")
            make_identity(nc, ident[:])
            ones_f = cpool.tile([128, 1], F32, tag="onesf")
            nc.vector.memset(ones_f[:], 1.0)
            ones = cpool.tile([128, 1], BF16, tag="ones")
            nc.vector.tensor_copy(ones[:], ones_f[:])
            onesr_f = cpool.tile([1, 128], F32, tag="onesrf")
            nc.vector.memset(onesr_f[:], 1.0)
            ones_row = cpool.tile([1, 128], F32R, tag="onesr")
            nc.vector.tensor_copy(ones_row[:], onesr_f[:])


            qT = spool.tile([128, B, GLOC, NTT, 128], BF16, tag="qT")
            kT = spool.tile([128, B, NTT, 128], BF16, tag="kT")
            vS = spool.tile([128, B, NSB, 128], BF16, tag="vS")

            # ---------------- phase 1: projections + rope ----------------
            def emit_proj_tiles(b, tts, rots, pps, p1):
                ri_q = rope_idx[(b, 'q')]
                ri_k = rope_idx[(b, 'k')]
                for tt in tts:
                    xt = p1.tile([128, NDC, 128], BF16, tag="xt", bufs=6)
                    # contiguous tile-blocked read: (p, k, j)
                    in_ap = bass.AP(
                        xT_d.tensor,
                        (b * NTT + tt) * (128 * NDC * 128),
                        [[NDC * 128, 128], [128, NDC], [1, 128]],
                    )
                    nc.sync.dma_start(out=xt[:], in_=in_ap)
                    qkv = pps.tile([128, 512], F32, tag="qkv")
                    for k in range(NDC):
                        nc.tensor.matmul(qkv[:], xt[:, k, :], w_sb[:, k, :],
                                         start=(k == 0), stop=(k == NDC - 1))
                    # epilogue (ACT/DVE only): rms stats -> scale -> rope
                    accs = p1.tile([128, 4], F32, tag="accs", bufs=6)
                    for hd in range(3):
                        sq = p1.tile([128, 128], F32, tag="sq", bufs=2)
                        nc.scalar.activation(
                            sq[:], qkv[:, hd * 128:hd * 128 + 128],
                            mybir.ActivationFunctionType.Square,
                            accum_out=accs[:, hd:hd + 1])
                    rmsv = p1.tile([128, 4], F32, tag="rmsv", bufs=6)
                    nc.scalar.activation(
                        rmsv[:, 0:3], accs[:, 0:3],
                        mybir.ActivationFunctionType.Sqrt,
                        bias=float(EPS), scale=1.0 / H)
                    rcp = p1.tile([128, 4], F32, tag="rcp", bufs=6)
                    nc.vector.reciprocal(rcp[:, 0:3], rmsv[:, 0:3])
                    qs = p1.tile([128, 3, 128], F32, tag="qs", bufs=3)
                    for hd in range(3):
                        nc.vector.tensor_scalar(
                            out=qs[:, hd, :],
                            in0=qkv[:, hd * 128:hd * 128 + 128],
                            scalar1=rcp[:, hd:hd + 1], scalar2=None,
                            op0=mybir.AluOpType.mult)
                    rot = p1.tile([128, 3, 128], BF16, tag="rot", bufs=NTT + 2,
                                  name=f"rot_{b}_{tt}")
                    # rope: q pair in one [128,2,64] op set, k separately
                    CAq = rope_sb[:, ri_q, 0, tt, :].unsqueeze(1).broadcast_to([128, 2, 64])
                    SAq = rope_sb[:, ri_q, 1, tt, :].unsqueeze(1).broadcast_to([128, 2, 64])
                    CBq = rope_sb[:, ri_q, 2, tt, :].unsqueeze(1).broadcast_to([128, 2, 64])
                    SBq = rope_sb[:, ri_q, 3, tt, :].unsqueeze(1).broadcast_to([128, 2, 64])
                    q1 = qs[:, 0:2, 0:64]
                    q2 = qs[:, 0:2, 64:128]
                    t1 = p1.tile([128, 2, 64], F32, tag="t1", bufs=3)
                    t2 = p1.tile([128, 2, 64], F32, tag="t2", bufs=3)
                    nc.vector.tensor_mul(t1[:], q1, CAq)
                    nc.vector.tensor_mul(t2[:], q2, SBq)
                    nc.vector.tensor_sub(rot[:, 0:2, 0:64], t1[:], t2[:])
                    nc.vector.tensor_mul(t1[:], q2, CBq)
                    nc.vector.tensor_mul(t2[:], q1, SAq)
                    nc.vector.tensor_add(rot[:, 0:2, 64:128], t1[:], t2[:])
                    CAk = rope_sb[:, ri_k, 0, tt, :]
                    SAk = rope_sb[:, ri_k, 1, tt, :]
                    CBk = rope_sb[:, ri_k, 2, tt, :]
                    SBk = rope_sb[:, ri_k, 3, tt, :]
                    k1 = qs[:, 2, 0:64]
                    k2 = qs[:, 2, 64:128]
                    t3 = p1.tile([128, 64], F32, tag="t3", bufs=3)
                    t4 = p1.tile([128, 64], F32, tag="t4", bufs=3)
                    nc.vector.tensor_mul(t3[:], k1, CAk)
                    nc.vector.tensor_mul(t4[:], k2, SBk)
                    nc.vector.tensor_sub(rot[:, 2, 0:64], t3[:], t4[:])
                    nc.vector.tensor_mul(t3[:], k2, CBk)
                    nc.vector.tensor_mul(t4[:], k1, SAk)
                    nc.vector.tensor_add(rot[:, 2, 64:128], t3[:], t4[:])
                    rots[tt] = rot
                    # v: plain copy [t,h] -> bf16 store (ACT; Pool can't read PSUM)
                    nc.scalar.copy(vS[:, b, tt, :], qkv[:, 384:512])

            def emit_transposes(b, tts, rots, tps):
                # k tiles first (next attention chunk needs kT for every block)
                for tt in tts:
                    ptr = tps.tile([128, 128], BF16, tag="ptr")
                    nc.tensor.transpose(ptr[:], rots[tt][:, 2, :], ident[:])
                    nc.any.tensor_copy(kT[:, b, tt, :], ptr[:])
                for g in range(GLOC):
                    for tt in tts:
                        ptr = tps.tile([128, 128], BF16, tag="ptr")
                        nc.tensor.transpose(ptr[:], rots[tt][:, g, :], ident[:])
                        nc.any.tensor_copy(qT[:, b, g, tt, :], ptr[:])

            # ------------- phase 2: flipped attention per (b, tc) -------------
            def emit_attn(b, tcc, lgp, opp, smp, p2):
                blist = blocks[b][tcc]
                o_ps = [opp.tile([128, 512], F32, tag="o",
                                 name=f"o_{b}_{tcc}_{g}") for g in range(GLOC)]
                # bf16 accumulator: all-2-byte operands put the DVE adds in
                # 2x perf mode, halving the chain that paces big chunks'
                # AllGather launches (<=16 sequential adds keep the rounding
                # drift ~0.5% on the softmax denominator)
                esum = [p2.tile([128, 512], BF16, tag="esum", bufs=4,
                                name=f"es_{b}_{tcc}_{g}") for g in range(GLOC)]
                qrhs = [qT[:, b, g, 4 * tcc:4 * tcc + 4, :].rearrange("p a b -> p (a b)")
                        for g in range(GLOC)]
                nblk = len(blist)
                DEPTH = 2
                exs = [None] * nblk
                for i in range(nblk + DEPTH):
                    if i < nblk:
                        sb, bidx = blist[i]
                        cur = []
                        for g in range(GLOC):
                            lg = lgp.tile([128, 512], F32, tag="lg")
                            nc.tensor.matmul(lg[:], kT[:, b, sb, :], qrhs[g],
                                             start=True, stop=True)
                            ex = p2.tile([128, 512], BF16, tag="ex", bufs=8)
                            nc.scalar.activation(
                                ex[:], lg[:],
                                mybir.ActivationFunctionType.Exp,
                                bias=0.0, scale=HSCALE)
                            if bidx is not None:
                                # mask applied post-exp as a 0/1 multiply
                                # (cheaper than a PSUM bias add; exp of
                                # unmasked logits is bounded per the
                                # structural check)
                                nc.vector.tensor_mul(ex[:], ex[:],
                                                     bias_sb[:, bidx, :])
                            # softmax partial sums accumulate on DVE (no PE)
                            if i == 0:
                                nc.vector.tensor_copy(esum[g][:], ex[:])
                            else:
                                nc.vector.tensor_add(esum[g][:], esum[g][:], ex[:])
                            cur.append(ex)
                        exs[i] = (sb, cur)
                    if i >= DEPTH:
                        sbp, exp_prev = exs[i - DEPTH]
                        first, last = (i - DEPTH == 0), (i - DEPTH == nblk - 1)
                        for g in range(GLOC):
                            nc.tensor.matmul(o_ps[g][:], vS[:, b, sbp, :],
                                             exp_prev[g][:],
                                             start=first, stop=last)
                for g in range(GLOC):
                    # one partition-reduction matmul per (chunk, g) replaces
                    # the per-block ones-matmul streams
                    s_ps = smp.tile([1, 512], F32, tag="s")
                    nc.tensor.matmul(s_ps[:], ones[:], esum[g][:],
                                     start=True, stop=True)
                    o_sb = p2.tile([128, 512], F32, tag="osb2", bufs=2)
                    nc.scalar.copy(o_sb[:], o_ps[g][:])
                    rec = p2.tile([1, 512], F32, tag="rec", bufs=2)
                    nc.vector.reciprocal_approx_fast(rec[:], s_ps[:])
                    rcb = p2.tile([128, 512], F32, tag="rcb", bufs=2)
                    nc.gpsimd.partition_broadcast(rcb[:], rec[:])
                    otn = p2.tile([128, 512], BF16, tag="otn", bufs=2)
                    nc.vector.tensor_mul(otn[:], o_sb[:], rcb[:])
                    dst = ag_in1[b][tcc].ap()[g * 128:(g + 1) * 128, :]
                    nc.gpsimd.dma_start(out=dst, in_=otn[:])
                nc.gpsimd.collective_compute(
                    "AllGather", mybir.AluOpType.bypass,
                    replica_groups=[list(range(NCORES))],
                    ins=[ag_in1[b][tcc].ap()],
                    outs=[ag_out1[b][tcc].ap()],
                )

            # ---------------- phase 3: o_proj (D-sharded) ----------------
            def emit_oproj(b, tcc, p3p, p3):
                outp = [p3p.tile([128, 512], F32, tag="op",
                                 name=f"op_{b}_{tcc}_{dh}") for dh in range(2)]
                for half in range(2):
                    oin = p3.tile([128, 8, 512], BF16, tag="oin", bufs=4)
                    src = bass.AP(
                        ag_out1[b][tcc].ap().tensor,
                        half * 8 * 128 * 512,
                        [[512, 128], [128 * 512, 8], [1, 512]])
                    nc.sync.dma_start(out=oin[:], in_=src)
                    for j in range(8):
                        nh = half * 8 + j
                        for dh in range(2):
                            nc.tensor.matmul(
                                outp[dh][:],
                                ow_sb[:, nh, dh * 128:dh * 128 + 128],
                                oin[:, j, :], start=(nh == 0), stop=(nh == N - 1))
                for dh in range(2):
                    osb = p3.tile([128, 512], F32, tag="osb", bufs=3)
                    nc.scalar.copy(osb[:], outp[dh][:])
                    nc.scalar.dma_start(
                        out=out_d[dh, :, b * T + tcc * 512: b * T + (tcc + 1) * 512],
                        in_=osb[:])

            # =================== schedule ===================
            # SBUF pools are shared across the whole kernel (continuous
            # x-tile prefetch, no region-reuse barriers); only the scarce
            # PSUM banks are scoped per section. b1's projection is split
            # 12+4 with attention chunks 0-2 in between, so their
            # AllGathers drain ~60us earlier and the serialized CC pipe
            # overlaps compute instead of forming the tail.
            rots = {}
            with tc.tile_pool(name="p1", bufs=3) as p1, \
                 tc.tile_pool(name="p2", bufs=4) as p2, \
                 tc.tile_pool(name="p3sb", bufs=3) as p3:
                with tc.tile_pool(name="pjA", bufs=3, space="PSUM") as pps, \
                     tc.tile_pool(name="tpA", bufs=3, space="PSUM") as tps:
                    emit_proj_tiles(0, range(NTT), rots, pps, p1)
                    emit_transposes(0, range(NTT), rots, tps)
                with tc.tile_pool(name="lgA", bufs=3, space="PSUM") as lgp, \
                     tc.tile_pool(name="opA", bufs=2, space="PSUM") as opp, \
                     tc.tile_pool(name="smA", bufs=1, space="PSUM") as smp:
                    for tcc in range(NTC):
                        emit_attn(0, tcc, lgp, opp, smp, p2)
                # o_w load off the critical path (needed from first o_proj)
                nc.gpsimd.dma_start(out=ow_sb[:], in_=bass.AP(
                    ow_d.tensor, 0, [[N * DLOC, 128], [1, N * DLOC]]))
                with tc.tile_pool(name="p3ps", bufs=2, space="PSUM") as p3p:
                    with tc.tile_pool(name="pjB", bufs=3, space="PSUM") as pps, \
                         tc.tile_pool(name="tpB", bufs=2, space="PSUM") as tps:
                        emit_proj_tiles(1, range(12), rots, pps, p1)
                        emit_transposes(1, range(12), rots, tps)
                    emit_oproj(0, 0, p3p, p3)
                    with tc.tile_pool(name="lgB", bufs=3, space="PSUM") as lgp, \
                         tc.tile_pool(name="opB", bufs=2, space="PSUM") as opp, \
                         tc.tile_pool(name="smB", bufs=1, space="PSUM") as smp:
                        for tcc in (0, 1, 2):
                            emit_attn(1, tcc, lgp, opp, smp, p2)
                    with tc.tile_pool(name="pjC", bufs=3, space="PSUM") as pps, \
                         tc.tile_pool(name="tpC", bufs=2, space="PSUM") as tps:
                        emit_proj_tiles(1, range(12, 16), rots, pps, p1)
                        emit_transposes(1, range(12, 16), rots, tps)
                    emit_oproj(0, 1, p3p, p3)
                    with tc.tile_pool(name="lgC", bufs=3, space="PSUM") as lgp, \
                         tc.tile_pool(name="opC", bufs=2, space="PSUM") as opp, \
                         tc.tile_pool(name="smC", bufs=1, space="PSUM") as smp:
                        emit_attn(1, 3, lgp, opp, smp, p2)
                    # drain: ready chunks first, then b1's in AllGather-
                    # completion order (the DMA queue is in-order)
                    for bx, tx in ((0, 2), (0, 3), (1, 0), (1, 1), (1, 2), (1, 3)):
                        emit_oproj(bx, tx, p3p, p3)

    nc.compile()

    in_maps = []
    for c in range(NCORES):
        in_maps.append({
            "xT": xT_bf,
            "w_all": w_all[c],
            "o_w": o_w_bf[c],
            "rope": rope_sets,
            "bias": bias_np,
        })
    trace = bool(os.environ.get("BASS_TRACE"))
    res = run_bass_kernel_spmd(nc, in_maps, core_ids=list(range(NCORES)),
                               trace=trace)
    global LAST_RES
    LAST_RES = res
    return res


def kernel(x, q_w, k_w, v_w, o_w, q_norm_w, k_norm_w, k_cache, v_cache,
           segment_ids, start_ind, cur_ind, right_pads):
    x = np.asarray(x, dtype=np.float32)
    q_w = np.asarray(q_w, dtype=np.float32)
    k_w = np.asarray(k_w, dtype=np.float32)
    v_w = np.asarray(v_w, dtype=np.float32)
    o_w = np.asarray(o_w, dtype=np.float32)
    q_norm_w = np.asarray(q_norm_w, dtype=np.float32)
    k_norm_w = np.asarray(k_norm_w, dtype=np.float32)
    segment_ids = np.asarray(segment_ids)
    start_ind = np.asarray(start_ind)
    ci = int(np.asarray(cur_ind))

    mask, sin, cos = _host_mask_and_rope(x, q_norm_w, k_norm_w, segment_ids,
                                         start_ind, ci)

    structural = (
        x.shape == (B, T, D) and ci == 0 and S == T
        and bool(mask.any(axis=-1).all())
        and float(np.sqrt(H) * np.abs(q_norm_w).max() * np.abs(k_norm_w).max()) < 80.0
    )
    blocks = bias_np = None
    if structural:
        blocks, bias_np = _block_structure(mask)
        structural = blocks is not None
    if not structural:
        return _numpy_reference(x, q_w, k_w, v_w, o_w, q_norm_w, k_norm_w,
                                k_cache, v_cache, segment_ids, start_ind, ci)

    # ---- host-side data prep ----
    import ml_dtypes
    BF = ml_dtypes.bfloat16
    # tile-blocked layout [B, tt, p, k, j] = x[b, tt*128+j, k*128+p] so each
    # (b,tt) projection tile is one fat contiguous DMA (4KB per partition)
    xT_blk = np.ascontiguousarray(
        x.reshape(B, NTT, 128, NDC, 128).transpose(0, 1, 4, 3, 2))
    xT_bf = xT_blk.astype(BF)

    w_all = []
    o_w_bf = []
    ow_flat = o_w.reshape(N * H, D)
    for c in range(NCORES):
        wc = np.concatenate([
            q_w[:, 2 * c:2 * c + 2, :].reshape(D, 2 * H),
            k_w[:, c, :],
            v_w[:, c, :],
        ], axis=1)                                             # [D, 512]
        w_all.append(np.ascontiguousarray(
            wc.reshape(NDC, 128, 512).transpose(1, 0, 2)
        ).reshape(128, NDC * 512).astype(BF))
        oc = ow_flat[:, c * DLOC:(c + 1) * DLOC]               # [2048, 256]
        o_w_bf.append(np.ascontiguousarray(
            oc.reshape(N, 128, DLOC).transpose(1, 0, 2)
        ).reshape(128, N * DLOC).astype(BF))

    # rope tables fused with norm weights: CA, SA, CB, SB each [T, 64]
    rope_sets = []
    rope_key = {}
    rope_idx = {}
    for b in range(B):
        for kind, w in (('q', q_norm_w), ('k', k_norm_w)):
            CA = cos[b] * w[None, :64]
            SA = sin[b] * w[None, :64]
            CB = cos[b] * w[None, 64:]
            SB = sin[b] * w[None, 64:]
            arr = np.stack([CA, SA, CB, SB]).astype(np.float32)  # [4, T, 64]
            key = arr.tobytes()
            if key not in rope_key:
                rope_key[key] = len(rope_sets)
                rope_sets.append(arr.reshape(4, NTT, 128, 64))
            rope_idx[(b, kind)] = rope_key[key]
    rope_sets = np.stack(rope_sets)                            # [R, 4, NTT, 128, 64]
    rope_sets = np.ascontiguousarray(
        rope_sets.transpose(3, 0, 1, 2, 4)).reshape(
        128, len(rope_key) * 4 * NTT * 64)

    # bias tiles become 0/1 multiplicative masks (applied post-exp)
    mask01 = (bias_np == 0).astype(BF)
    res = _build_and_run(xT_bf, w_all, o_w_bf, rope_sets, rope_idx,
                         mask01, blocks)

    out = np.empty((B, T, D), dtype=np.float32)
    for c in range(NCORES):
        oc = np.asarray(res.results[c]["out"])                 # [2, 128, B*T]
        oc = oc.reshape(DLOC, B, T).transpose(1, 2, 0)         # [B, T, 256]
        out[:, :, c * DLOC:(c + 1) * DLOC] = oc
    return out


if __name__ == "__main__":
    import reference
    inputs = {k: np.asarray(v) for k, v in reference.setup_inputs().items()}
    got = kernel(**inputs)
    print("kernel output", got.shape, got.dtype)


# revision 74
# speedup vs baseline: 1.0386x; 1.0386x over previous
"""8-core TP attention kernel for Trainium2 (Bass/Tile).

Problem: B=2, T=S=2048, D=2048, N=16 q-heads, KH=8 kv-heads, H=128.
Sharding: TP over heads. Core c owns q-heads {2c, 2c+1}, kv-head c, and the
D-output slice [256c, 256(c+1)) of o_proj. Per-head attention outputs are
AllGathered (bf16) across cores; o_proj is sharded on its output dim so the
host just concatenates the 8 output slices.

Changes vs the 485us fp32r baseline (now ~390-400us measured):
- bf16 end-to-end on the matmul path (x, qkv weights, kT/qT/vS, exp, mask):
  same PE stream rate as fp32r but ~2x faster LDWEIGHTS and half the x DMA.
- softmax sums no longer burn a PE stream per block: exp tiles accumulate on
  DVE (esum, f32r) and a single ones-matmul per (b,chunk,g) reduces across
  partitions.
- the causal mask is a 0/1 multiply on the exp output (bf16, DVE) instead of
  a K_MASK bias add on the PSUM logits.
- three parallel startup DMA streams: qkv weights on sync, x tiles on
  gpsimd, rope/bias on scalar; o_w loads after b0's sections. w/rope/o_w
  use per-partition-contiguous DRAM layouts (one fat descriptor per
  partition instead of thousands of scattered ~1KB reads), which removed
  the ~16us startup stall entirely.
- b1's projection is split 12+4 with attention chunks 0-2 in between, so
  their AllGathers drain ~60us earlier on the serialized CC pipe and overlap
  compute instead of forming the tail. SBUF pools are shared across the
  whole kernel (continuous x-tile prefetch); only PSUM pools are scoped per
  section. o_proj chunks for b0 fill section boundaries; the rest drain at
  the end in AllGather-completion order, covering the last AllGather.
"""
import sys
import os

sys.path.insert(0, "/opt/trn_rl_repo")

# Provide antenv.axon_hooks (missing from the read-only antenv package on
# PYTHONPATH) so bass_utils can capture NTFF profiles under axon when
# trace=True. Degrades to a None hook (trace skipped) when the .so lacks the
# profile symbols.
if "antenv.axon_hooks" not in sys.modules:
    import types as _types

    _mod = _types.ModuleType("antenv.axon_hooks")

    def _default_ntff_hook():
        import contextlib
        import ctypes

        so_path = "/opt/axon/libaxon_pjrt.so"
        if not os.path.exists(so_path):
            return None
        lib = ctypes.CDLL(so_path)
        if not hasattr(lib, "axon_start_nrt_profile"):
            return None
        lib.axon_start_nrt_profile.argtypes = [
            ctypes.POINTER(ctypes.c_int64), ctypes.c_size_t]
        lib.axon_start_nrt_profile.restype = ctypes.c_int64
        lib.axon_stop_nrt_profile.argtypes = [ctypes.c_char_p]
        lib.axon_stop_nrt_profile.restype = ctypes.c_int64

        @contextlib.contextmanager
        def _hook(output_dir, device_ids):
            import jax
            jax.devices()
            if device_ids:
                ids = (ctypes.c_int64 * len(device_ids))(*device_ids)
                rc = lib.axon_start_nrt_profile(ids, len(device_ids))
            else:
                rc = lib.axon_start_nrt_profile(None, 0)
            if rc != 0:
                raise RuntimeError(f"axon_start_nrt_profile rc={rc}")
            try:
                yield
            finally:
                n = lib.axon_stop_nrt_profile(str(output_dir).encode())
                if n < 0:
                    raise RuntimeError(f"axon_stop_nrt_profile rc={n}")
                print(f"profile: {n} file(s) written to {output_dir}")

        return _hook

    _mod._HOOK = None

    def _set_hook(hook, _m=_mod):
        _m._HOOK = hook

    def _get_hook(_m=_mod):
        if _m._HOOK is None:
            _m._HOOK = _default_ntff_hook()
        return _m._HOOK

    _mod.set_axon_ntff_profile_hook = _set_hook
    _mod.get_axon_ntff_profile_hook = _get_hook
    sys.modules["antenv.axon_hooks"] = _mod
    try:
        import antenv as _antenv
        _antenv.axon_hooks = _mod
    except ImportError:
        pass

import numpy as np

B, T, D = 2, 2048, 2048
N, KH, H = 16, 8, 128
S = 2048
EPS = 1e-6
ROPE_THETA = 1000000.0
K_MASK = -0.7 * float(np.finfo(np.float32).max)
NCORES = 8
GLOC = N // NCORES        # 2 local q heads
DLOC = D // NCORES        # 256 output cols per core
NTT = T // 128            # 16 t-tiles
NTC = T // 512            # 4 t-chunks
NDC = D // 128            # 16 d-chunks
NSB = S // 128            # 16 s-blocks
HSCALE = float(H) ** -0.5
MAX_BIAS = 8


def _num_left_pad(seg):
    return np.sum(np.cumsum(seg != 0, axis=-1) == 0, axis=-1).astype(np.int32)


def _positions_from_segment_ids(seg):
    t = seg.shape[1]
    pos = np.arange(t, dtype=np.int32)[None, :] - np.argmax(seg, axis=1)[:, None]
    return np.where(seg != 0, pos, 2 ** 30)


def _host_mask_and_rope(x, q_norm_w, k_norm_w, segment_ids, start_ind, cur_ind):
    """Reproduce the reference mask / positions / rope tables in numpy."""
    b, t = segment_ids.shape
    s = S
    start = np.where(start_ind < 0, _num_left_pad(segment_ids), start_ind).astype(np.int64)
    pos = _positions_from_segment_ids(segment_ids).astype(np.int64) + int(cur_ind)

    fraction = np.arange(0, H, 2, dtype=np.float32) / np.float32(H)
    inv_freq = (1.0 / (np.float32(ROPE_THETA) ** fraction)).astype(np.float32)
    sinusoid = (pos.astype(np.float32)[:, :, None] * inv_freq[None, None, :]).astype(np.float32)
    sin, cos = np.sin(sinusoid).astype(np.float32), np.cos(sinusoid).astype(np.float32)

    q_pos = int(cur_ind) + np.arange(t, dtype=np.int64)[None, :] - start[:, None]
    ts_ = np.arange(s, dtype=np.int64)
    kv_seg = (ts_[None, :] >= start[:, None]) & (ts_[None, :] < int(cur_ind) + t)
    k_pos = ts_[None, :] - start[:, None]
    causal = k_pos[:, None, :] <= q_pos[:, :, None]
    seg_mask = kv_seg[:, None, :].astype(segment_ids.dtype) == segment_ids[:, :, None]
    final_mask = causal & seg_mask  # [B, T, S]
    return final_mask, sin, cos


def _numpy_reference(x, q_w, k_w, v_w, o_w, q_norm_w, k_norm_w, k_cache, v_cache,
                     segment_ids, start_ind, cur_ind):
    """Exact-ish numpy fallback (used only for non-structural inputs)."""
    def rms_norm(v, w):
        rms = np.sqrt(np.mean(v.astype(np.float32) ** 2, axis=-1, keepdims=True) + EPS)
        return (w * v / rms).astype(v.dtype)

    mask, sin, cos = _host_mask_and_rope(x, q_norm_w, k_norm_w, segment_ids,
                                         start_ind, cur_ind)

    q = rms_norm(np.einsum('BTD,DNH->BTNH', x, q_w), q_norm_w)
    k = rms_norm(np.einsum('BSD,DKH->BSKH', x, k_w), k_norm_w)
    v = np.einsum('BSD,DKH->BSKH', x, v_w)

    def rope(z):
        h = z.shape[-1] // 2
        z1, z2 = z[..., :h], z[..., h:]
        s_, c_ = sin[:, :, None, :], cos[:, :, None, :]
        return np.concatenate([z1 * c_ - z2 * s_, z2 * c_ + z1 * s_], axis=-1).astype(z.dtype)

    q, k = rope(q), rope(k)
    kc = np.array(k_cache)
    vc = np.array(v_cache)
    ci = int(cur_ind)
    kc[:, ci:ci + T] = k
    vc[:, ci:ci + T] = v

    b, t = x.shape[0], x.shape[1]
    qg = q.reshape(b, t, KH, N // KH, H)
    logits = np.einsum('BTHGD,BSHD->BHGTS', qg, kc) * HSCALE
    logits = np.where(mask[:, None, None, :, :], logits, np.float32(K_MASK))
    m = logits.max(axis=-1, keepdims=True)
    e = np.exp(logits - m)
    attn = (e / e.sum(axis=-1, keepdims=True)).astype(np.float32)
    o = np.einsum('BHGTS,BSHD->BTHGD', attn, vc).reshape(b, t, N, H)
    return np.einsum('BTNH,NHD->BTD', o, o_w).astype(np.float32)


def _block_structure(mask):
    """Classify [s,t] blocks of 128x512 per (b, tc).

    Returns blocks[b][tc] = list of (sb, bias_idx|None) and bias tiles
    [nbias, 128, 512] f32 additive masks (0 valid / K_MASK invalid), or None
    if the structure is unsupported (fallback needed).
    """
    bias_tiles = []
    bias_map = {}
    blocks = []
    for b in range(B):
        per_b = []
        for tcc in range(NTC):
            sub = mask[b, tcc * 512:(tcc + 1) * 512, :]  # [512 t, S]
            lst = []
            for sb in range(NSB):
                blk = sub[:, sb * 128:(sb + 1) * 128].T  # [128 s, 512 t]
                if not blk.any():
                    continue
                if blk.all():
                    lst.append((sb, None))
                    continue
                key = blk.tobytes()
                if key not in bias_map:
                    bias_map[key] = len(bias_tiles)
                    bias_tiles.append(np.where(blk, np.float32(0), np.float32(K_MASK)))
                lst.append((sb, bias_map[key]))
            per_b.append(lst)
        blocks.append(per_b)
    if len(bias_tiles) == 0:
        bias_tiles.append(np.zeros((128, 512), np.float32))
    if len(bias_tiles) > MAX_BIAS:
        return None, None
    return blocks, np.stack(bias_tiles)


LAST_RES = None


def _build_and_run(xT_bf, w_all, o_w_bf, rope_sets, rope_idx, bias_np, blocks):
    import ml_dtypes
    import concourse.bass as bass
    import concourse.mybir as mybir
    import concourse.tile as tile
    from concourse import bacc
    from concourse.bass_utils import run_bass_kernel_spmd
    from concourse.masks import make_identity

    F32 = mybir.dt.float32
    F32R = mybir.dt.float32r
    BF16 = mybir.dt.bfloat16
    NBIAS = bias_np.shape[0]
    NRSETS = rope_sets.shape[1] // (4 * NTT * 64)

    nc = bacc.Bacc("TRN2", target_bir_lowering=False, debug=False, num_devices=NCORES)
    eps_t = nc.alloc_sbuf_tensor("const-eps", [128, 1], F32)
    nc.gpsimd.memset(eps_t.ap(), float(EPS))
    nc.const_aps.aps[(F32, float(EPS))] = eps_t.ap()

    # ---- external I/O ----
    xT_d = nc.dram_tensor("xT", [B, NTT, 128, NDC, 128], BF16, kind="ExternalInput").ap()
    w_d = nc.dram_tensor("w_all", [128, NDC * 512], BF16, kind="ExternalInput").ap()
    ow_d = nc.dram_tensor("o_w", [128, N * DLOC], BF16, kind="ExternalInput").ap()
    rope_d = nc.dram_tensor("rope", [128, NRSETS * 4 * NTT * 64], F32,
                            kind="ExternalInput").ap()
    bias_d = nc.dram_tensor("bias", [NBIAS, 128, 512], BF16, kind="ExternalInput").ap()
    out_d = nc.dram_tensor("out", [2, 128, B * T], F32, kind="ExternalOutput").ap()

    # ---- collective buffers: per (b,tc) chunks so the AllGather stream
    # starts as soon as the first chunk's attention lands and stays
    # pipelined with compute ----
    ag_in1 = [[nc.dram_tensor(f"agin_{b}_{tcc}", [GLOC * 128, 512], BF16)
               for tcc in range(NTC)] for b in range(B)]
    ag_out1 = [[nc.dram_tensor(f"agout_{b}_{tcc}", [N * 128, 512], BF16,
                               addr_space="Shared") for tcc in range(NTC)]
               for b in range(B)]

    with tile.TileContext(nc) as tc:
        with tc.tile_pool(name="const", bufs=1) as cpool, \
             tc.tile_pool(name="store", bufs=1) as spool:
            # persistent tiles; w on the sync queue, x tiles on the gpsimd
            # queue, rope/bias on the scalar queue — three parallel streams
            # so the first projection matmul starts ~15us in
            # per-partition-contiguous layout: 128 fat descriptors instead
            # of 2048 scattered 1KB reads, so w lands ~12us earlier
            w_sb = cpool.tile([128, NDC, 512], BF16, tag="w")
            nc.sync.dma_start(out=w_sb[:], in_=bass.AP(
                w_d.tensor, 0, [[NDC * 512, 128], [1, NDC * 512]]))
            ow_sb = cpool.tile([128, N, DLOC], BF16, tag="ow")
            rope_sb = cpool.tile([128, NRSETS, 4, NTT, 64], F32, tag="rope")
            nc.scalar.dma_start(out=rope_sb[:], in_=bass.AP(
                rope_d.tensor, 0,
                [[NRSETS * 4 * NTT * 64, 128], [1, NRSETS * 4 * NTT * 64]]))
            bias_sb = cpool.tile([128, NBIAS, 512], BF16, tag="bias")
            nc.scalar.dma_start(out=bias_sb[:], in_=bass.AP(
                bias_d.tensor, 0, [[512, 128], [128 * 512, NBIAS], [1, 512]]))
            ident = cpool.tile([128, 128], BF16, tag="ident")
            make_identity(nc, ident[:])
            ones_f = cpool.tile([128, 1], F32, tag="onesf")
            nc.vector.memset(ones_f[:], 1.0)
            ones = cpool.tile([128, 1], BF16, tag="ones")
            nc.vector.tensor_copy(ones[:], ones_f[:])
            onesr_f = cpool.tile([1, 128], F32, tag="onesrf")
            nc.vector.memset(onesr_f[:], 1.0)
            ones_row = cpool.tile([1, 128], F32R, tag="onesr")
            nc.vector.tensor_copy(ones_row[:], onesr_f[:])


            qT = spool.tile([128, B, GLOC, NTT, 128], BF16, tag="qT")
            kT = spool.tile([128, B, NTT, 128], BF16, tag="kT")
            vS = spool.tile([128, B, NSB, 128], BF16, tag="vS")

            # ---------------- phase 1: projections + rope ----------------
            def emit_proj_tiles(b, tts, rots, pps, p1):
                ri_q = rope_idx[(b, 'q')]
                ri_k = rope_idx[(b, 'k')]
                for tt in tts:
                    xt = p1.tile([128, NDC, 128], BF16, tag="xt", bufs=6)
                    # contiguous tile-blocked read: (p, k, j)
                    in_ap = bass.AP(
                        xT_d.tensor,
                        (b * NTT + tt) * (128 * NDC * 128),
                        [[NDC * 128, 128], [128, NDC], [1, 128]],
                    )
                    nc.gpsimd.dma_start(out=xt[:], in_=in_ap)
                    qkv = pps.tile([128, 512], F32, tag="qkv")
                    for k in range(NDC):
                        nc.tensor.matmul(qkv[:], xt[:, k, :], w_sb[:, k, :],
                                         start=(k == 0), stop=(k == NDC - 1))
                    # epilogue (ACT/DVE only): rms stats -> scale -> rope
                    accs = p1.tile([128, 4], F32, tag="accs", bufs=6)
                    for hd in range(3):
                        sq = p1.tile([128, 128], F32, tag="sq", bufs=2)
                        nc.scalar.activation(
                            sq[:], qkv[:, hd * 128:hd * 128 + 128],
                            mybir.ActivationFunctionType.Square,
                            accum_out=accs[:, hd:hd + 1])
                    rmsv = p1.tile([128, 4], F32, tag="rmsv", bufs=6)
                    nc.scalar.activation(
                        rmsv[:, 0:3], accs[:, 0:3],
                        mybir.ActivationFunctionType.Sqrt,
                        bias=float(EPS), scale=1.0 / H)
                    rcp = p1.tile([128, 4], F32, tag="rcp", bufs=6)
                    nc.vector.reciprocal(rcp[:, 0:3], rmsv[:, 0:3])
                    qs = p1.tile([128, 3, 128], F32, tag="qs", bufs=3)
                    for hd in range(3):
                        nc.vector.tensor_scalar(
                            out=qs[:, hd, :],
                            in0=qkv[:, hd * 128:hd * 128 + 128],
                            scalar1=rcp[:, hd:hd + 1], scalar2=None,
                            op0=mybir.AluOpType.mult)
                    rot = p1.tile([128, 3, 128], BF16, tag="rot", bufs=NTT + 2,
                                  name=f"rot_{b}_{tt}")
                    # rope: q pair in one [128,2,64] op set, k separately
                    CAq = rope_sb[:, ri_q, 0, tt, :].unsqueeze(1).broadcast_to([128, 2, 64])
                    SAq = rope_sb[:, ri_q, 1, tt, :].unsqueeze(1).broadcast_to([128, 2, 64])
                    CBq = rope_sb[:, ri_q, 2, tt, :].unsqueeze(1).broadcast_to([128, 2, 64])
                    SBq = rope_sb[:, ri_q, 3, tt, :].unsqueeze(1).broadcast_to([128, 2, 64])
                    q1 = qs[:, 0:2, 0:64]
                    q2 = qs[:, 0:2, 64:128]
                    t1 = p1.tile([128, 2, 64], F32, tag="t1", bufs=3)
                    t2 = p1.tile([128, 2, 64], F32, tag="t2", bufs=3)
                    nc.vector.tensor_mul(t1[:], q1, CAq)
                    nc.vector.tensor_mul(t2[:], q2, SBq)
                    nc.vector.tensor_sub(rot[:, 0:2, 0:64], t1[:], t2[:])
                    nc.vector.tensor_mul(t1[:], q2, CBq)
                    nc.vector.tensor_mul(t2[:], q1, SAq)
                    nc.vector.tensor_add(rot[:, 0:2, 64:128], t1[:], t2[:])
                    CAk = rope_sb[:, ri_k, 0, tt, :]
                    SAk = rope_sb[:, ri_k, 1, tt, :]
                    CBk = rope_sb[:, ri_k, 2, tt, :]
                    SBk = rope_sb[:, ri_k, 3, tt, :]
                    k1 = qs[:, 2, 0:64]
                    k2 = qs[:, 2, 64:128]
                    t3 = p1.tile([128, 64], F32, tag="t3", bufs=3)
                    t4 = p1.tile([128, 64], F32, tag="t4", bufs=3)
                    nc.vector.tensor_mul(t3[:], k1, CAk)
                    nc.vector.tensor_mul(t4[:], k2, SBk)
                    nc.vector.tensor_sub(rot[:, 2, 0:64], t3[:], t4[:])
                    nc.vector.tensor_mul(t3[:], k2, CBk)
                    nc.vector.tensor_mul(t4[:], k1, SAk)
                    nc.vector.tensor_add(rot[:, 2, 64:128], t3[:], t4[:])
                    rots[tt] = rot
                    # v: plain copy [t,h] -> bf16 store (ACT; Pool can't read PSUM)
                    nc.scalar.copy(vS[:, b, tt, :], qkv[:, 384:512])

            def emit_transposes(b, tts, rots, tps):
                # k tiles first (next attention chunk needs kT for every block)
                for tt in tts:
                    ptr = tps.tile([128, 128], BF16, tag="ptr")
                    nc.tensor.transpose(ptr[:], rots[tt][:, 2, :], ident[:])
                    nc.any.tensor_copy(kT[:, b, tt, :], ptr[:])
                for g in range(GLOC):
                    for tt in tts:
                        ptr = tps.tile([128, 128], BF16, tag="ptr")
                        nc.tensor.transpose(ptr[:], rots[tt][:, g, :], ident[:])
                        nc.any.tensor_copy(qT[:, b, g, tt, :], ptr[:])

            # ------------- phase 2: flipped attention per (b, tc) -------------
            def emit_attn(b, tcc, lgp, opp, smp, p2):
                blist = blocks[b][tcc]
                o_ps = [opp.tile([128, 512], F32, tag="o",
                                 name=f"o_{b}_{tcc}_{g}") for g in range(GLOC)]
                # bf16 accumulator: all-2-byte operands put the DVE adds in
                # 2x perf mode, halving the chain that paces big chunks'
                # AllGather launches (<=16 sequential adds keep the rounding
                # drift ~0.5% on the softmax denominator)
                esum = [p2.tile([128, 512], BF16, tag="esum", bufs=4,
                                name=f"es_{b}_{tcc}_{g}") for g in range(GLOC)]
                qrhs = [qT[:, b, g, 4 * tcc:4 * tcc + 4, :].rearrange("p a b -> p (a b)")
                        for g in range(GLOC)]
                nblk = len(blist)
                DEPTH = 2
                exs = [None] * nblk
                for i in range(nblk + DEPTH):
                    if i < nblk:
                        sb, bidx = blist[i]
                        cur = []
                        for g in range(GLOC):
                            lg = lgp.tile([128, 512], F32, tag="lg")
                            nc.tensor.matmul(lg[:], kT[:, b, sb, :], qrhs[g],
                                             start=True, stop=True)
                            ex = p2.tile([128, 512], BF16, tag="ex", bufs=8)
                            nc.scalar.activation(
                                ex[:], lg[:],
                                mybir.ActivationFunctionType.Exp,
                                bias=0.0, scale=HSCALE)
                            if bidx is not None:
                                # mask applied post-exp as a 0/1 multiply
                                # (cheaper than a PSUM bias add; exp of
                                # unmasked logits is bounded per the
                                # structural check)
                                nc.vector.tensor_mul(ex[:], ex[:],
                                                     bias_sb[:, bidx, :])
                            # softmax partial sums accumulate on DVE (no PE)
                            if i == 0:
                                nc.vector.tensor_copy(esum[g][:], ex[:])
                            else:
                                nc.vector.tensor_add(esum[g][:], esum[g][:], ex[:])
                            cur.append(ex)
                        exs[i] = (sb, cur)
                    if i >= DEPTH:
                        sbp, exp_prev = exs[i - DEPTH]
                        first, last = (i - DEPTH == 0), (i - DEPTH == nblk - 1)
                        for g in range(GLOC):
                            nc.tensor.matmul(o_ps[g][:], vS[:, b, sbp, :],
                                             exp_prev[g][:],
                                             start=first, stop=last)
                for g in range(GLOC):
                    # one partition-reduction matmul per (chunk, g) replaces
                    # the per-block ones-matmul streams
                    s_ps = smp.tile([1, 512], F32, tag="s")
                    nc.tensor.matmul(s_ps[:], ones[:], esum[g][:],
                                     start=True, stop=True)
                    o_sb = p2.tile([128, 512], F32, tag="osb2", bufs=2)
                    nc.scalar.copy(o_sb[:], o_ps[g][:])
                    rec = p2.tile([1, 512], F32, tag="rec", bufs=2)
                    nc.vector.reciprocal_approx_fast(rec[:], s_ps[:])
                    rcb = p2.tile([128, 512], F32, tag="rcb", bufs=2)
                    nc.gpsimd.partition_broadcast(rcb[:], rec[:])
                    otn = p2.tile([128, 512], BF16, tag="otn", bufs=2)
                    nc.vector.tensor_mul(otn[:], o_sb[:], rcb[:])
                    dst = ag_in1[b][tcc].ap()[g * 128:(g + 1) * 128, :]
                    nc.gpsimd.dma_start(out=dst, in_=otn[:])
                nc.gpsimd.collective_compute(
                    "AllGather", mybir.AluOpType.bypass,
                    replica_groups=[list(range(NCORES))],
                    ins=[ag_in1[b][tcc].ap()],
                    outs=[ag_out1[b][tcc].ap()],
                )

            # ---------------- phase 3: o_proj (D-sharded) ----------------
            def emit_oproj(b, tcc, p3p, p3):
                outp = [p3p.tile([128, 512], F32, tag="op",
                                 name=f"op_{b}_{tcc}_{dh}") for dh in range(2)]
                for half in range(2):
                    oin = p3.tile([128, 8, 512], BF16, tag="oin", bufs=4)
                    src = bass.AP(
                        ag_out1[b][tcc].ap().tensor,
                        half * 8 * 128 * 512,
                        [[512, 128], [128 * 512, 8], [1, 512]])
                    nc.sync.dma_start(out=oin[:], in_=src)
                    for j in range(8):
                        nh = half * 8 + j
                        for dh in range(2):
                            nc.tensor.matmul(
                                outp[dh][:],
                                ow_sb[:, nh, dh * 128:dh * 128 + 128],
                                oin[:, j, :], start=(nh == 0), stop=(nh == N - 1))
                for dh in range(2):
                    osb = p3.tile([128, 512], F32, tag="osb", bufs=3)
                    nc.scalar.copy(osb[:], outp[dh][:])
                    nc.scalar.dma_start(
                        out=out_d[dh, :, b * T + tcc * 512: b * T + (tcc + 1) * 512],
                        in_=osb[:])

            # =================== schedule ===================
            # SBUF pools are shared across the whole kernel (continuous
            # x-tile prefetch, no region-reuse barriers); only the scarce
            # PSUM banks are scoped per section. b1's projection is split
            # 12+4 with attention chunks 0-2 in between, so their
            # AllGathers drain ~60us earlier and the serialized CC pipe
            # overlaps compute instead of forming the tail.
            rots = {}
            with tc.tile_pool(name="p1", bufs=3) as p1, \
                 tc.tile_pool(name="p2", bufs=4) as p2, \
                 tc.tile_pool(name="p3sb", bufs=3) as p3:
                with tc.tile_pool(name="pjA", bufs=3, space="PSUM") as pps, \
                     tc.tile_pool(name="tpA", bufs=3, space="PSUM") as tps:
                    emit_proj_tiles(0, range(NTT), rots, pps, p1)
                    emit_transposes(0, range(NTT), rots, tps)
                with tc.tile_pool(name="lgA", bufs=3, space="PSUM") as lgp, \
                     tc.tile_pool(name="opA", bufs=2, space="PSUM") as opp, \
                     tc.tile_pool(name="smA", bufs=1, space="PSUM") as smp:
                    for tcc in range(NTC):
                        emit_attn(0, tcc, lgp, opp, smp, p2)
                # o_w load off the critical path (needed from first o_proj)
                nc.gpsimd.dma_start(out=ow_sb[:], in_=bass.AP(
                    ow_d.tensor, 0, [[N * DLOC, 128], [1, N * DLOC]]))
                with tc.tile_pool(name="p3ps", bufs=2, space="PSUM") as p3p:
                    with tc.tile_pool(name="pjB", bufs=3, space="PSUM") as pps, \
                         tc.tile_pool(name="tpB", bufs=2, space="PSUM") as tps:
                        emit_proj_tiles(1, range(12), rots, pps, p1)
                        emit_transposes(1, range(12), rots, tps)
                    emit_oproj(0, 0, p3p, p3)
                    with tc.tile_pool(name="lgB", bufs=3, space="PSUM") as lgp, \
                         tc.tile_pool(name="opB", bufs=2, space="PSUM") as opp, \
                         tc.tile_pool(name="smB", bufs=1, space="PSUM") as smp:
                        for tcc in (0, 1, 2):
                            emit_attn(1, tcc, lgp, opp, smp, p2)
                    with tc.tile_pool(name="pjC", bufs=3, space="PSUM") as pps, \
                         tc.tile_pool(name="tpC", bufs=2, space="PSUM") as tps:
                        emit_proj_tiles(1, range(12, 16), rots, pps, p1)
                        emit_transposes(1, range(12, 16), rots, tps)
                    emit_oproj(0, 1, p3p, p3)
                    with tc.tile_pool(name="lgC", bufs=3, space="PSUM") as lgp, \
                         tc.tile_pool(name="opC", bufs=2, space="PSUM") as opp, \
                         tc.tile_pool(name="smC", bufs=1, space="PSUM") as smp:
                        emit_attn(1, 3, lgp, opp, smp, p2)
                    # drain: ready chunks first, then b1's in AllGather-
                    # completion order (the DMA queue is in-order)
                    for bx, tx in ((0, 2), (0, 3), (1, 0), (1, 1), (1, 2), (1, 3)):
                        emit_oproj(bx, tx, p3p, p3)

    nc.compile()

    in_maps = []
    for c in range(NCORES):
        in_maps.append({
            "xT": xT_bf,
            "w_all": w_all[c],
            "o_w": o_w_bf[c],
            "rope": rope_sets,
            "bias": bias_np,
        })
    trace = bool(os.environ.get("BASS_TRACE"))
    res = run_bass_kernel_spmd(nc, in_maps, core_ids=list(range(NCORES)),
                               trace=trace)
    global LAST_RES
    LAST_RES = res
    return res


def kernel(x, q_w, k_w, v_w, o_w, q_norm_w, k_norm_w, k_cache, v_cache,
           segment_ids, start_ind, cur_ind, right_pads):
    x = np.asarray(x, dtype=np.float32)
    q_w = np.asarray(q_w, dtype=np.float32)
    k_w = np.asarray(k_w, dtype=np.float32)
    v_w = np.asarray(v_w, dtype=np.float32)
    o_w = np.asarray(o_w, dtype=np.float32)
    q_norm_w = np.asarray(q_norm_w, dtype=np.float32)
    k_norm_w = np.asarray(k_norm_w, dtype=np.float32)
    segment_ids = np.asarray(segment_ids)
    start_ind = np.asarray(start_ind)
    ci = int(np.asarray(cur_ind))

    mask, sin, cos = _host_mask_and_rope(x, q_norm_w, k_norm_w, segment_ids,
                                         start_ind, ci)

    structural = (
        x.shape == (B, T, D) and ci == 0 and S == T
        and bool(mask.any(axis=-1).all())
        and float(np.sqrt(H) * np.abs(q_norm_w).max() * np.abs(k_norm_w).max()) < 80.0
    )
    blocks = bias_np = None
    if structural:
        blocks, bias_np = _block_structure(mask)
        structural = blocks is not None
    if not structural:
        return _numpy_reference(x, q_w, k_w, v_w, o_w, q_norm_w, k_norm_w,
                                k_cache, v_cache, segment_ids, start_ind, ci)

    # ---- host-side data prep ----
    import ml_dtypes
    BF = ml_dtypes.bfloat16
    # tile-blocked layout [B, tt, p, k, j] = x[b, tt*128+j, k*128+p] so each
    # (b,tt) projection tile is one fat contiguous DMA (4KB per partition)
    xT_blk = np.ascontiguousarray(
        x.reshape(B, NTT, 128, NDC, 128).transpose(0, 1, 4, 3, 2))
    xT_bf = xT_blk.astype(BF)

    w_all = []
    o_w_bf = []
    ow_flat = o_w.reshape(N * H, D)
    for c in range(NCORES):
        wc = np.concatenate([
            q_w[:, 2 * c:2 * c + 2, :].reshape(D, 2 * H),
            k_w[:, c, :],
            v_w[:, c, :],
        ], axis=1)                                             # [D, 512]
        w_all.append(np.ascontiguousarray(
            wc.reshape(NDC, 128, 512).transpose(1, 0, 2)
        ).reshape(128, NDC * 512).astype(BF))
        oc = ow_flat[:, c * DLOC:(c + 1) * DLOC]               # [2048, 256]
        o_w_bf.append(np.ascontiguousarray(
            oc.reshape(N, 128, DLOC).transpose(1, 0, 2)
        ).reshape(128, N * DLOC).astype(BF))

    # rope tables fused with norm weights: CA, SA, CB, SB each [T, 64]
    rope_sets = []
    rope_key = {}
    rope_idx = {}
    for b in range(B):
        for kind, w in (('q', q_norm_w), ('k', k_norm_w)):
            CA = cos[b] * w[None, :64]
            SA = sin[b] * w[None, :64]
            CB = cos[b] * w[None, 64:]
            SB = sin[b] * w[None, 64:]
            arr = np.stack([CA, SA, CB, SB]).astype(np.float32)  # [4, T, 64]
            key = arr.tobytes()
            if key not in rope_key:
                rope_key[key] = len(rope_sets)
                rope_sets.append(arr.reshape(4, NTT, 128, 64))
            rope_idx[(b, kind)] = rope_key[key]
    rope_sets = np.stack(rope_sets)                            # [R, 4, NTT, 128, 64]
    rope_sets = np.ascontiguousarray(
        rope_sets.transpose(3, 0, 1, 2, 4)).reshape(
        128, len(rope_key) * 4 * NTT * 64)

    # bias tiles become 0/1 multiplicative masks (applied post-exp)
    mask01 = (bias_np == 0).astype(BF)
    res = _build_and_run(xT_bf, w_all, o_w_bf, rope_sets, rope_idx,
                         mask01, blocks)

    out = np.empty((B, T, D), dtype=np.float32)
    for c in range(NCORES):
        oc = np.asarray(res.results[c]["out"])                 # [2, 128, B*T]
        oc = oc.reshape(DLOC, B, T).transpose(1, 2, 0)         # [B, T, 256]
        out[:, :, c * DLOC:(c + 1) * DLOC] = oc
    return out


if __name__ == "__main__":
    import reference
    inputs = {k: np.asarray(v) for k, v in reference.setup_inputs().items()}
    got = kernel(**inputs)
    print("kernel output", got.shape, got.dtype)


# revision 75
# speedup vs baseline: 1.0500x; 1.0109x over previous
"""8-core TP attention kernel for Trainium2 (Bass/Tile).

Problem: B=2, T=S=2048, D=2048, N=16 q-heads, KH=8 kv-heads, H=128.
Sharding: TP over heads. Core c owns q-heads {2c, 2c+1}, kv-head c, and the
D-output slice [256c, 256(c+1)) of o_proj. Per-head attention outputs are
AllGathered (bf16) across cores; o_proj is sharded on its output dim so the
host just concatenates the 8 output slices.

Changes vs the 485us fp32r baseline (now ~390-400us measured):
- bf16 end-to-end on the matmul path (x, qkv weights, kT/qT/vS, exp, mask):
  same PE stream rate as fp32r but ~2x faster LDWEIGHTS and half the x DMA.
- softmax sums no longer burn a PE stream per block: exp tiles accumulate on
  DVE (esum, f32r) and a single ones-matmul per (b,chunk,g) reduces across
  partitions.
- the causal mask is a 0/1 multiply on the exp output (bf16, DVE) instead of
  a K_MASK bias add on the PSUM logits.
- three parallel startup DMA streams: qkv weights on sync, x tiles on
  gpsimd, rope/bias on scalar; o_w loads after b0's sections. w/rope/o_w
  use per-partition-contiguous DRAM layouts (one fat descriptor per
  partition instead of thousands of scattered ~1KB reads), which removed
  the ~16us startup stall entirely.
- b1's projection is split 12+4 with attention chunks 0-2 in between, so
  their AllGathers drain ~60us earlier on the serialized CC pipe and overlap
  compute instead of forming the tail. SBUF pools are shared across the
  whole kernel (continuous x-tile prefetch); only PSUM pools are scoped per
  section. o_proj chunks for b0 fill section boundaries; the rest drain at
  the end in AllGather-completion order, covering the last AllGather.
"""
import sys
import os

sys.path.insert(0, "/opt/trn_rl_repo")

# Provide antenv.axon_hooks (missing from the read-only antenv package on
# PYTHONPATH) so bass_utils can capture NTFF profiles under axon when
# trace=True. Degrades to a None hook (trace skipped) when the .so lacks the
# profile symbols.
if "antenv.axon_hooks" not in sys.modules:
    import types as _types

    _mod = _types.ModuleType("antenv.axon_hooks")

    def _default_ntff_hook():
        import contextlib
        import ctypes

        so_path = "/opt/axon/libaxon_pjrt.so"
        if not os.path.exists(so_path):
            return None
        lib = ctypes.CDLL(so_path)
        if not hasattr(lib, "axon_start_nrt_profile"):
            return None
        lib.axon_start_nrt_profile.argtypes = [
            ctypes.POINTER(ctypes.c_int64), ctypes.c_size_t]
        lib.axon_start_nrt_profile.restype = ctypes.c_int64
        lib.axon_stop_nrt_profile.argtypes = [ctypes.c_char_p]
        lib.axon_stop_nrt_profile.restype = ctypes.c_int64

        @contextlib.contextmanager
        def _hook(output_dir, device_ids):
            import jax
            jax.devices()
            if device_ids:
                ids = (ctypes.c_int64 * len(device_ids))(*device_ids)
                rc = lib.axon_start_nrt_profile(ids, len(device_ids))
            else:
                rc = lib.axon_start_nrt_profile(None, 0)
            if rc != 0:
                raise RuntimeError(f"axon_start_nrt_profile rc={rc}")
            try:
                yield
            finally:
                n = lib.axon_stop_nrt_profile(str(output_dir).encode())
                if n < 0:
                    raise RuntimeError(f"axon_stop_nrt_profile rc={n}")
                print(f"profile: {n} file(s) written to {output_dir}")

        return _hook

    _mod._HOOK = None

    def _set_hook(hook, _m=_mod):
        _m._HOOK = hook

    def _get_hook(_m=_mod):
        if _m._HOOK is None:
            _m._HOOK = _default_ntff_hook()
        return _m._HOOK

    _mod.set_axon_ntff_profile_hook = _set_hook
    _mod.get_axon_ntff_profile_hook = _get_hook
    sys.modules["antenv.axon_hooks"] = _mod
    try:
        import antenv as _antenv
        _antenv.axon_hooks = _mod
    except ImportError:
        pass

import numpy as np

B, T, D = 2, 2048, 2048
N, KH, H = 16, 8, 128
S = 2048
EPS = 1e-6
ROPE_THETA = 1000000.0
K_MASK = -0.7 * float(np.finfo(np.float32).max)
NCORES = 8
GLOC = N // NCORES        # 2 local q heads
DLOC = D // NCORES        # 256 output cols per core
NTT = T // 128            # 16 t-tiles
NTC = T // 512            # 4 t-chunks
NDC = D // 128            # 16 d-chunks
NSB = S // 128            # 16 s-blocks
HSCALE = float(H) ** -0.5
MAX_BIAS = 8


def _num_left_pad(seg):
    return np.sum(np.cumsum(seg != 0, axis=-1) == 0, axis=-1).astype(np.int32)


def _positions_from_segment_ids(seg):
    t = seg.shape[1]
    pos = np.arange(t, dtype=np.int32)[None, :] - np.argmax(seg, axis=1)[:, None]
    return np.where(seg != 0, pos, 2 ** 30)


def _host_mask_and_rope(x, q_norm_w, k_norm_w, segment_ids, start_ind, cur_ind):
    """Reproduce the reference mask / positions / rope tables in numpy."""
    b, t = segment_ids.shape
    s = S
    start = np.where(start_ind < 0, _num_left_pad(segment_ids), start_ind).astype(np.int64)
    pos = _positions_from_segment_ids(segment_ids).astype(np.int64) + int(cur_ind)

    fraction = np.arange(0, H, 2, dtype=np.float32) / np.float32(H)
    inv_freq = (1.0 / (np.float32(ROPE_THETA) ** fraction)).astype(np.float32)
    sinusoid = (pos.astype(np.float32)[:, :, None] * inv_freq[None, None, :]).astype(np.float32)
    sin, cos = np.sin(sinusoid).astype(np.float32), np.cos(sinusoid).astype(np.float32)

    q_pos = int(cur_ind) + np.arange(t, dtype=np.int64)[None, :] - start[:, None]
    ts_ = np.arange(s, dtype=np.int64)
    kv_seg = (ts_[None, :] >= start[:, None]) & (ts_[None, :] < int(cur_ind) + t)
    k_pos = ts_[None, :] - start[:, None]
    causal = k_pos[:, None, :] <= q_pos[:, :, None]
    seg_mask = kv_seg[:, None, :].astype(segment_ids.dtype) == segment_ids[:, :, None]
    final_mask = causal & seg_mask  # [B, T, S]
    return final_mask, sin, cos


def _numpy_reference(x, q_w, k_w, v_w, o_w, q_norm_w, k_norm_w, k_cache, v_cache,
                     segment_ids, start_ind, cur_ind):
    """Exact-ish numpy fallback (used only for non-structural inputs)."""
    def rms_norm(v, w):
        rms = np.sqrt(np.mean(v.astype(np.float32) ** 2, axis=-1, keepdims=True) + EPS)
        return (w * v / rms).astype(v.dtype)

    mask, sin, cos = _host_mask_and_rope(x, q_norm_w, k_norm_w, segment_ids,
                                         start_ind, cur_ind)

    q = rms_norm(np.einsum('BTD,DNH->BTNH', x, q_w), q_norm_w)
    k = rms_norm(np.einsum('BSD,DKH->BSKH', x, k_w), k_norm_w)
    v = np.einsum('BSD,DKH->BSKH', x, v_w)

    def rope(z):
        h = z.shape[-1] // 2
        z1, z2 = z[..., :h], z[..., h:]
        s_, c_ = sin[:, :, None, :], cos[:, :, None, :]
        return np.concatenate([z1 * c_ - z2 * s_, z2 * c_ + z1 * s_], axis=-1).astype(z.dtype)

    q, k = rope(q), rope(k)
    kc = np.array(k_cache)
    vc = np.array(v_cache)
    ci = int(cur_ind)
    kc[:, ci:ci + T] = k
    vc[:, ci:ci + T] = v

    b, t = x.shape[0], x.shape[1]
    qg = q.reshape(b, t, KH, N // KH, H)
    logits = np.einsum('BTHGD,BSHD->BHGTS', qg, kc) * HSCALE
    logits = np.where(mask[:, None, None, :, :], logits, np.float32(K_MASK))
    m = logits.max(axis=-1, keepdims=True)
    e = np.exp(logits - m)
    attn = (e / e.sum(axis=-1, keepdims=True)).astype(np.float32)
    o = np.einsum('BHGTS,BSHD->BTHGD', attn, vc).reshape(b, t, N, H)
    return np.einsum('BTNH,NHD->BTD', o, o_w).astype(np.float32)


def _block_structure(mask):
    """Classify [s,t] blocks of 128x512 per (b, tc).

    Returns blocks[b][tc] = list of (sb, bias_idx|None) and bias tiles
    [nbias, 128, 512] f32 additive masks (0 valid / K_MASK invalid), or None
    if the structure is unsupported (fallback needed).
    """
    bias_tiles = []
    bias_map = {}
    blocks = []
    for b in range(B):
        per_b = []
        for tcc in range(NTC):
            sub = mask[b, tcc * 512:(tcc + 1) * 512, :]  # [512 t, S]
            lst = []
            for sb in range(NSB):
                blk = sub[:, sb * 128:(sb + 1) * 128].T  # [128 s, 512 t]
                if not blk.any():
                    continue
                if blk.all():
                    lst.append((sb, None))
                    continue
                key = blk.tobytes()
                if key not in bias_map:
                    bias_map[key] = len(bias_tiles)
                    bias_tiles.append(np.where(blk, np.float32(0), np.float32(K_MASK)))
                lst.append((sb, bias_map[key]))
            per_b.append(lst)
        blocks.append(per_b)
    if len(bias_tiles) == 0:
        bias_tiles.append(np.zeros((128, 512), np.float32))
    if len(bias_tiles) > MAX_BIAS:
        return None, None
    return blocks, np.stack(bias_tiles)


LAST_RES = None


def _build_and_run(xT_bf, w_all, o_w_bf, rope_sets, rope_idx, bias_np, blocks):
    import ml_dtypes
    import concourse.bass as bass
    import concourse.mybir as mybir
    import concourse.tile as tile
    from concourse import bacc
    from concourse.bass_utils import run_bass_kernel_spmd
    from concourse.masks import make_identity

    F32 = mybir.dt.float32
    F32R = mybir.dt.float32r
    BF16 = mybir.dt.bfloat16
    NBIAS = bias_np.shape[0]
    NRSETS = rope_sets.shape[1] // (4 * NTT * 64)

    nc = bacc.Bacc("TRN2", target_bir_lowering=False, debug=False, num_devices=NCORES)
    eps_t = nc.alloc_sbuf_tensor("const-eps", [128, 1], F32)
    nc.gpsimd.memset(eps_t.ap(), float(EPS))
    nc.const_aps.aps[(F32, float(EPS))] = eps_t.ap()

    # ---- external I/O ----
    xT_d = nc.dram_tensor("xT", [B, NTT, 128, NDC, 128], BF16, kind="ExternalInput").ap()
    w_d = nc.dram_tensor("w_all", [128, NDC * 512], BF16, kind="ExternalInput").ap()
    ow_d = nc.dram_tensor("o_w", [128, N * DLOC], BF16, kind="ExternalInput").ap()
    rope_d = nc.dram_tensor("rope", [128, NRSETS * 4 * NTT * 64], F32,
                            kind="ExternalInput").ap()
    bias_d = nc.dram_tensor("bias", [NBIAS, 128, 512], BF16, kind="ExternalInput").ap()
    out_d = nc.dram_tensor("out", [2, 128, B * T], F32, kind="ExternalOutput").ap()

    # ---- collective buffers: per (b,tc) chunks so the AllGather stream
    # starts as soon as the first chunk's attention lands and stays
    # pipelined with compute ----
    ag_in1 = [[nc.dram_tensor(f"agin_{b}_{tcc}", [GLOC * 128, 512], BF16)
               for tcc in range(NTC)] for b in range(B)]
    ag_out1 = [[nc.dram_tensor(f"agout_{b}_{tcc}", [N * 128, 512], BF16,
                               addr_space="Shared") for tcc in range(NTC)]
               for b in range(B)]

    with tile.TileContext(nc) as tc:
        with tc.tile_pool(name="const", bufs=1) as cpool, \
             tc.tile_pool(name="store", bufs=1) as spool:
            # persistent tiles; w on the sync queue, x tiles on the gpsimd
            # queue, rope/bias on the scalar queue — three parallel streams
            # so the first projection matmul starts ~15us in
            # per-partition-contiguous layout: 128 fat descriptors instead
            # of 2048 scattered 1KB reads, so w lands ~12us earlier
            w_sb = cpool.tile([128, NDC, 512], BF16, tag="w")
            nc.sync.dma_start(out=w_sb[:], in_=bass.AP(
                w_d.tensor, 0, [[NDC * 512, 128], [1, NDC * 512]]))
            ow_sb = cpool.tile([128, N, DLOC], BF16, tag="ow")
            rope_sb = cpool.tile([128, NRSETS, 4, NTT, 64], F32, tag="rope")
            nc.scalar.dma_start(out=rope_sb[:], in_=bass.AP(
                rope_d.tensor, 0,
                [[NRSETS * 4 * NTT * 64, 128], [1, NRSETS * 4 * NTT * 64]]))
            bias_sb = cpool.tile([128, NBIAS, 512], BF16, tag="bias")
            nc.scalar.dma_start(out=bias_sb[:], in_=bass.AP(
                bias_d.tensor, 0, [[512, 128], [128 * 512, NBIAS], [1, 512]]))
            ident = cpool.tile([128, 128], BF16, tag="ident")
            make_identity(nc, ident[:])
            ones_f = cpool.tile([128, 1], F32, tag="onesf")
            nc.vector.memset(ones_f[:], 1.0)
            ones = cpool.tile([128, 1], BF16, tag="ones")
            nc.vector.tensor_copy(ones[:], ones_f[:])
            onesr_f = cpool.tile([1, 128], F32, tag="onesrf")
            nc.vector.memset(onesr_f[:], 1.0)
            ones_row = cpool.tile([1, 128], F32R, tag="onesr")
            nc.vector.tensor_copy(ones_row[:], onesr_f[:])


            qT = spool.tile([128, B, GLOC, NTT, 128], BF16, tag="qT")
            kT = spool.tile([128, B, NTT, 128], BF16, tag="kT")
            vS = spool.tile([128, B, NSB, 128], BF16, tag="vS")

            # ---------------- phase 1: projections + rope ----------------
            def emit_proj_tiles(b, tts, rots, pps, p1):
                ri_q = rope_idx[(b, 'q')]
                ri_k = rope_idx[(b, 'k')]
                for tt in tts:
                    xt = p1.tile([128, NDC, 128], BF16, tag="xt", bufs=6)
                    # contiguous tile-blocked read: (p, k, j)
                    in_ap = bass.AP(
                        xT_d.tensor,
                        (b * NTT + tt) * (128 * NDC * 128),
                        [[NDC * 128, 128], [128, NDC], [1, 128]],
                    )
                    nc.gpsimd.dma_start(out=xt[:], in_=in_ap)
                    qkv = pps.tile([128, 512], F32, tag="qkv")
                    for k in range(NDC):
                        nc.tensor.matmul(qkv[:], xt[:, k, :], w_sb[:, k, :],
                                         start=(k == 0), stop=(k == NDC - 1))
                    # epilogue (ACT/DVE only): rms stats -> scale -> rope
                    accs = p1.tile([128, 4], F32, tag="accs", bufs=6)
                    for hd in range(3):
                        sq = p1.tile([128, 128], F32, tag="sq", bufs=2)
                        nc.scalar.activation(
                            sq[:], qkv[:, hd * 128:hd * 128 + 128],
                            mybir.ActivationFunctionType.Square,
                            accum_out=accs[:, hd:hd + 1])
                    rmsv = p1.tile([128, 4], F32, tag="rmsv", bufs=6)
                    nc.scalar.activation(
                        rmsv[:, 0:3], accs[:, 0:3],
                        mybir.ActivationFunctionType.Sqrt,
                        bias=float(EPS), scale=1.0 / H)
                    rcp = p1.tile([128, 4], F32, tag="rcp", bufs=6)
                    nc.vector.reciprocal(rcp[:, 0:3], rmsv[:, 0:3])
                    qs = p1.tile([128, 3, 128], F32, tag="qs", bufs=3)
                    for hd in range(3):
                        nc.vector.tensor_scalar(
                            out=qs[:, hd, :],
                            in0=qkv[:, hd * 128:hd * 128 + 128],
                            scalar1=rcp[:, hd:hd + 1], scalar2=None,
                            op0=mybir.AluOpType.mult)
                    rot = p1.tile([128, 3, 128], BF16, tag="rot", bufs=NTT + 2,
                                  name=f"rot_{b}_{tt}")
                    # rope: q pair in one [128,2,64] op set, k separately
                    CAq = rope_sb[:, ri_q, 0, tt, :].unsqueeze(1).broadcast_to([128, 2, 64])
                    SAq = rope_sb[:, ri_q, 1, tt, :].unsqueeze(1).broadcast_to([128, 2, 64])
                    CBq = rope_sb[:, ri_q, 2, tt, :].unsqueeze(1).broadcast_to([128, 2, 64])
                    SBq = rope_sb[:, ri_q, 3, tt, :].unsqueeze(1).broadcast_to([128, 2, 64])
                    q1 = qs[:, 0:2, 0:64]
                    q2 = qs[:, 0:2, 64:128]
                    t1 = p1.tile([128, 2, 64], F32, tag="t1", bufs=3)
                    t2 = p1.tile([128, 2, 64], F32, tag="t2", bufs=3)
                    nc.vector.tensor_mul(t1[:], q1, CAq)
                    nc.vector.tensor_mul(t2[:], q2, SBq)
                    nc.vector.tensor_sub(rot[:, 0:2, 0:64], t1[:], t2[:])
                    nc.vector.tensor_mul(t1[:], q2, CBq)
                    nc.vector.tensor_mul(t2[:], q1, SAq)
                    nc.vector.tensor_add(rot[:, 0:2, 64:128], t1[:], t2[:])
                    CAk = rope_sb[:, ri_k, 0, tt, :]
                    SAk = rope_sb[:, ri_k, 1, tt, :]
                    CBk = rope_sb[:, ri_k, 2, tt, :]
                    SBk = rope_sb[:, ri_k, 3, tt, :]
                    k1 = qs[:, 2, 0:64]
                    k2 = qs[:, 2, 64:128]
                    t3 = p1.tile([128, 64], F32, tag="t3", bufs=3)
                    t4 = p1.tile([128, 64], F32, tag="t4", bufs=3)
                    nc.vector.tensor_mul(t3[:], k1, CAk)
                    nc.vector.tensor_mul(t4[:], k2, SBk)
                    nc.vector.tensor_sub(rot[:, 2, 0:64], t3[:], t4[:])
                    nc.vector.tensor_mul(t3[:], k2, CBk)
                    nc.vector.tensor_mul(t4[:], k1, SAk)
                    nc.vector.tensor_add(rot[:, 2, 64:128], t3[:], t4[:])
                    rots[tt] = rot
                    # v: plain copy [t,h] -> bf16 store (ACT; Pool can't read PSUM)
                    nc.scalar.copy(vS[:, b, tt, :], qkv[:, 384:512])

            def emit_transposes(b, tts, rots, tps):
                # k tiles first (next attention chunk needs kT for every block)
                for tt in tts:
                    ptr = tps.tile([128, 128], BF16, tag="ptr")
                    nc.tensor.transpose(ptr[:], rots[tt][:, 2, :], ident[:])
                    nc.any.tensor_copy(kT[:, b, tt, :], ptr[:])
                for g in range(GLOC):
                    for tt in tts:
                        ptr = tps.tile([128, 128], BF16, tag="ptr")
                        nc.tensor.transpose(ptr[:], rots[tt][:, g, :], ident[:])
                        nc.any.tensor_copy(qT[:, b, g, tt, :], ptr[:])

            # ------------- phase 2: flipped attention per (b, tc) -------------
            def emit_attn(b, tcc, lgp, opp, smp, p2):
                blist = blocks[b][tcc]
                o_ps = [opp.tile([128, 512], F32, tag="o",
                                 name=f"o_{b}_{tcc}_{g}") for g in range(GLOC)]
                # bf16 accumulator: all-2-byte operands put the DVE adds in
                # 2x perf mode, halving the chain that paces big chunks'
                # AllGather launches (<=16 sequential adds keep the rounding
                # drift ~0.5% on the softmax denominator)
                esum = [p2.tile([128, 512], BF16, tag="esum", bufs=4,
                                name=f"es_{b}_{tcc}_{g}") for g in range(GLOC)]
                qrhs = [qT[:, b, g, 4 * tcc:4 * tcc + 4, :].rearrange("p a b -> p (a b)")
                        for g in range(GLOC)]
                nblk = len(blist)
                DEPTH = 2
                exs = [None] * nblk
                for i in range(nblk + DEPTH):
                    if i < nblk:
                        sb, bidx = blist[i]
                        cur = []
                        for g in range(GLOC):
                            lg = lgp.tile([128, 512], F32, tag="lg")
                            nc.tensor.matmul(lg[:], kT[:, b, sb, :], qrhs[g],
                                             start=True, stop=True)
                            ex = p2.tile([128, 512], BF16, tag="ex", bufs=8)
                            nc.scalar.activation(
                                ex[:], lg[:],
                                mybir.ActivationFunctionType.Exp,
                                bias=0.0, scale=HSCALE)
                            if bidx is not None:
                                # mask applied post-exp as a 0/1 multiply
                                # (cheaper than a PSUM bias add; exp of
                                # unmasked logits is bounded per the
                                # structural check)
                                nc.vector.tensor_mul(ex[:], ex[:],
                                                     bias_sb[:, bidx, :])
                            # softmax partial sums accumulate on DVE (no PE)
                            if i == 0:
                                nc.vector.tensor_copy(esum[g][:], ex[:])
                            else:
                                nc.vector.tensor_add(esum[g][:], esum[g][:], ex[:])
                            cur.append(ex)
                        exs[i] = (sb, cur)
                    if i >= DEPTH:
                        sbp, exp_prev = exs[i - DEPTH]
                        first, last = (i - DEPTH == 0), (i - DEPTH == nblk - 1)
                        for g in range(GLOC):
                            nc.tensor.matmul(o_ps[g][:], vS[:, b, sbp, :],
                                             exp_prev[g][:],
                                             start=first, stop=last)
                for g in range(GLOC):
                    # one partition-reduction matmul per (chunk, g) replaces
                    # the per-block ones-matmul streams
                    s_ps = smp.tile([1, 512], F32, tag="s")
                    nc.tensor.matmul(s_ps[:], ones[:], esum[g][:],
                                     start=True, stop=True)
                    o_sb = p2.tile([128, 512], F32, tag="osb2", bufs=2)
                    nc.scalar.copy(o_sb[:], o_ps[g][:])
                    rec = p2.tile([1, 512], F32, tag="rec", bufs=2)
                    nc.vector.reciprocal_approx_fast(rec[:], s_ps[:])
                    rcb = p2.tile([128, 512], F32, tag="rcb", bufs=2)
                    nc.gpsimd.partition_broadcast(rcb[:], rec[:])
                    otn = p2.tile([128, 512], BF16, tag="otn", bufs=2)
                    nc.vector.tensor_mul(otn[:], o_sb[:], rcb[:])
                    dst = ag_in1[b][tcc].ap()[g * 128:(g + 1) * 128, :]
                    nc.gpsimd.dma_start(out=dst, in_=otn[:])
                nc.gpsimd.collective_compute(
                    "AllGather", mybir.AluOpType.bypass,
                    replica_groups=[list(range(NCORES))],
                    ins=[ag_in1[b][tcc].ap()],
                    outs=[ag_out1[b][tcc].ap()],
                )

            # ---------------- phase 3: o_proj (D-sharded) ----------------
            def emit_oproj(b, tcc, p3p, p3):
                outp = [p3p.tile([128, 512], F32, tag="op",
                                 name=f"op_{b}_{tcc}_{dh}") for dh in range(2)]
                for half in range(2):
                    oin = p3.tile([128, 8, 512], BF16, tag="oin", bufs=4)
                    src = bass.AP(
                        ag_out1[b][tcc].ap().tensor,
                        half * 8 * 128 * 512,
                        [[512, 128], [128 * 512, 8], [1, 512]])
                    nc.sync.dma_start(out=oin[:], in_=src)
                    for j in range(8):
                        nh = half * 8 + j
                        for dh in range(2):
                            nc.tensor.matmul(
                                outp[dh][:],
                                ow_sb[:, nh, dh * 128:dh * 128 + 128],
                                oin[:, j, :], start=(nh == 0), stop=(nh == N - 1))
                for dh in range(2):
                    osb = p3.tile([128, 512], F32, tag="osb", bufs=3)
                    nc.scalar.copy(osb[:], outp[dh][:])
                    nc.scalar.dma_start(
                        out=out_d[dh, :, b * T + tcc * 512: b * T + (tcc + 1) * 512],
                        in_=osb[:])

            # =================== schedule ===================
            # SBUF pools are shared across the whole kernel (continuous
            # x-tile prefetch, no region-reuse barriers); only the scarce
            # PSUM banks are scoped per section. b1's projection is split
            # 12+4 with attention chunks 0-2 in between, so their
            # AllGathers drain ~60us earlier and the serialized CC pipe
            # overlaps compute instead of forming the tail.
            rots = {}
            with tc.tile_pool(name="p1", bufs=3) as p1, \
                 tc.tile_pool(name="p2", bufs=4) as p2, \
                 tc.tile_pool(name="p3sb", bufs=3) as p3:
                with tc.tile_pool(name="pjA", bufs=3, space="PSUM") as pps, \
                     tc.tile_pool(name="tpA", bufs=3, space="PSUM") as tps:
                    emit_proj_tiles(0, range(NTT), rots, pps, p1)
                    emit_transposes(0, range(NTT), rots, tps)
                # tiny warmup AllGather emitted behind b0's x-tile DMAs on
                # the Pool queue: absorbs the first-collective setup cost
                # while the CC cores are idle
                warm_in = nc.dram_tensor("warm_in", [1, 64], BF16)
                warm_out = nc.dram_tensor("warm_out", [NCORES, 64], BF16,
                                          addr_space="Shared")
                nc.gpsimd.collective_compute(
                    "AllGather", mybir.AluOpType.bypass,
                    replica_groups=[list(range(NCORES))],
                    ins=[warm_in.ap()], outs=[warm_out.ap()])
                with tc.tile_pool(name="lgA", bufs=3, space="PSUM") as lgp, \
                     tc.tile_pool(name="opA", bufs=2, space="PSUM") as opp, \
                     tc.tile_pool(name="smA", bufs=1, space="PSUM") as smp:
                    for tcc in range(NTC):
                        emit_attn(0, tcc, lgp, opp, smp, p2)
                # o_w load off the critical path (needed from first o_proj)
                nc.gpsimd.dma_start(out=ow_sb[:], in_=bass.AP(
                    ow_d.tensor, 0, [[N * DLOC, 128], [1, N * DLOC]]))
                with tc.tile_pool(name="p3ps", bufs=2, space="PSUM") as p3p:
                    with tc.tile_pool(name="pjB", bufs=3, space="PSUM") as pps, \
                         tc.tile_pool(name="tpB", bufs=2, space="PSUM") as tps:
                        emit_proj_tiles(1, range(12), rots, pps, p1)
                        emit_transposes(1, range(12), rots, tps)
                    emit_oproj(0, 0, p3p, p3)
                    with tc.tile_pool(name="lgB", bufs=3, space="PSUM") as lgp, \
                         tc.tile_pool(name="opB", bufs=2, space="PSUM") as opp, \
                         tc.tile_pool(name="smB", bufs=1, space="PSUM") as smp:
                        for tcc in (0, 1, 2):
                            emit_attn(1, tcc, lgp, opp, smp, p2)
                    with tc.tile_pool(name="pjC", bufs=3, space="PSUM") as pps, \
                         tc.tile_pool(name="tpC", bufs=2, space="PSUM") as tps:
                        emit_proj_tiles(1, range(12, 16), rots, pps, p1)
                        emit_transposes(1, range(12, 16), rots, tps)
                    emit_oproj(0, 1, p3p, p3)
                    with tc.tile_pool(name="lgC", bufs=3, space="PSUM") as lgp, \
                         tc.tile_pool(name="opC", bufs=2, space="PSUM") as opp, \
                         tc.tile_pool(name="smC", bufs=1, space="PSUM") as smp:
                        emit_attn(1, 3, lgp, opp, smp, p2)
                    # drain: ready chunks first, then b1's in AllGather-
                    # completion order (the DMA queue is in-order)
                    for bx, tx in ((0, 2), (0, 3), (1, 0), (1, 1), (1, 2), (1, 3)):
                        emit_oproj(bx, tx, p3p, p3)

    nc.compile()

    in_maps = []
    for c in range(NCORES):
        in_maps.append({
            "xT": xT_bf,
            "w_all": w_all[c],
            "o_w": o_w_bf[c],
            "rope": rope_sets,
            "bias": bias_np,
        })
    trace = bool(os.environ.get("BASS_TRACE"))
    res = run_bass_kernel_spmd(nc, in_maps, core_ids=list(range(NCORES)),
                               trace=trace)
    global LAST_RES
    LAST_RES = res
    return res


def kernel(x, q_w, k_w, v_w, o_w, q_norm_w, k_norm_w, k_cache, v_cache,
           segment_ids, start_ind, cur_ind, right_pads):
    x = np.asarray(x, dtype=np.float32)
    q_w = np.asarray(q_w, dtype=np.float32)
    k_w = np.asarray(k_w, dtype=np.float32)
    v_w = np.asarray(v_w, dtype=np.float32)
    o_w = np.asarray(o_w, dtype=np.float32)
    q_norm_w = np.asarray(q_norm_w, dtype=np.float32)
    k_norm_w = np.asarray(k_norm_w, dtype=np.float32)
    segment_ids = np.asarray(segment_ids)
    start_ind = np.asarray(start_ind)
    ci = int(np.asarray(cur_ind))

    mask, sin, cos = _host_mask_and_rope(x, q_norm_w, k_norm_w, segment_ids,
                                         start_ind, ci)

    structural = (
        x.shape == (B, T, D) and ci == 0 and S == T
        and bool(mask.any(axis=-1).all())
        and float(np.sqrt(H) * np.abs(q_norm_w).max() * np.abs(k_norm_w).max()) < 80.0
    )
    blocks = bias_np = None
    if structural:
        blocks, bias_np = _block_structure(mask)
        structural = blocks is not None
    if not structural:
        return _numpy_reference(x, q_w, k_w, v_w, o_w, q_norm_w, k_norm_w,
                                k_cache, v_cache, segment_ids, start_ind, ci)

    # ---- host-side data prep ----
    import ml_dtypes
    BF = ml_dtypes.bfloat16
    # tile-blocked layout [B, tt, p, k, j] = x[b, tt*128+j, k*128+p] so each
    # (b,tt) projection tile is one fat contiguous DMA (4KB per partition)
    xT_blk = np.ascontiguousarray(
        x.reshape(B, NTT, 128, NDC, 128).transpose(0, 1, 4, 3, 2))
    xT_bf = xT_blk.astype(BF)

    w_all = []
    o_w_bf = []
    ow_flat = o_w.reshape(N * H, D)
    for c in range(NCORES):
        wc = np.concatenate([
            q_w[:, 2 * c:2 * c + 2, :].reshape(D, 2 * H),
            k_w[:, c, :],
            v_w[:, c, :],
        ], axis=1)                                             # [D, 512]
        w_all.append(np.ascontiguousarray(
            wc.reshape(NDC, 128, 512).transpose(1, 0, 2)
        ).reshape(128, NDC * 512).astype(BF))
        oc = ow_flat[:, c * DLOC:(c + 1) * DLOC]               # [2048, 256]
        o_w_bf.append(np.ascontiguousarray(
            oc.reshape(N, 128, DLOC).transpose(1, 0, 2)
        ).reshape(128, N * DLOC).astype(BF))

    # rope tables fused with norm weights: CA, SA, CB, SB each [T, 64]
    rope_sets = []
    rope_key = {}
    rope_idx = {}
    for b in range(B):
        for kind, w in (('q', q_norm_w), ('k', k_norm_w)):
            CA = cos[b] * w[None, :64]
            SA = sin[b] * w[None, :64]
            CB = cos[b] * w[None, 64:]
            SB = sin[b] * w[None, 64:]
            arr = np.stack([CA, SA, CB, SB]).astype(np.float32)  # [4, T, 64]
            key = arr.tobytes()
            if key not in rope_key:
                rope_key[key] = len(rope_sets)
                rope_sets.append(arr.reshape(4, NTT, 128, 64))
            rope_idx[(b, kind)] = rope_key[key]
    rope_sets = np.stack(rope_sets)                            # [R, 4, NTT, 128, 64]
    rope_sets = np.ascontiguousarray(
        rope_sets.transpose(3, 0, 1, 2, 4)).reshape(
        128, len(rope_key) * 4 * NTT * 64)

    # bias tiles become 0/1 multiplicative masks (applied post-exp)
    mask01 = (bias_np == 0).astype(BF)
    res = _build_and_run(xT_bf, w_all, o_w_bf, rope_sets, rope_idx,
                         mask01, blocks)

    out = np.empty((B, T, D), dtype=np.float32)
    for c in range(NCORES):
        oc = np.asarray(res.results[c]["out"])                 # [2, 128, B*T]
        oc = oc.reshape(DLOC, B, T).transpose(1, 2, 0)         # [B, T, 256]
        out[:, :, c * DLOC:(c + 1) * DLOC] = oc
    return out


if __name__ == "__main__":
    import reference
    inputs = {k: np.asarray(v) for k, v in reference.setup_inputs().items()}
    got = kernel(**inputs)
    print("kernel output", got.shape, got.dtype)
